# revision 27
# baseline (speedup 1.0000x reference)
"""Trainium2 Bass kernel for nn_CMABlock (CMA dual-stream transformer block).

Self-contained: hardcodes shapes/sharding. Data-parallel over batch across
8 NeuronCores (4 batch elements per core, both rgb/tir streams on each).

v2: head-paired stage 2 (opposite PE row-groups run concurrently), GPSIMD
softmax normalization, approx reciprocals, grouped LN-stats to avoid ACT
table-set thrashing, aggressive SBUF tag sharing.
"""
import sys
sys.path.insert(0, '/opt/trn_rl_repo')

import numpy as np
import ml_dtypes

import concourse.bass as bass
import concourse.tile as tile
from concourse import mybir
from concourse import bacc
from concourse import bass_utils
from concourse import library_config
from concourse.bass import ts, ds
from concourse.masks import make_identity

F32 = mybir.dt.float32
F32R = mybir.dt.float32r
BF16 = mybir.dt.bfloat16
AF = mybir.ActivationFunctionType
ALU = mybir.AluOpType

B, N, DIM, H = 32, 320, 768, 12
DH = DIM // H          # 64
T = 64                 # template tokens
S = N - T              # 256 search tokens
MATCH = 64
MLP_HID = 4 * DIM      # 3072
NCORES = 8
BL = B // NCORES       # 4 batch per core
FI = DIM // 128        # 6 feature chunks
TP = [128, 128, 64]    # token chunk sizes (320)
SCALE = DH ** -0.5     # 0.125
MSC = MATCH ** -0.5    # 0.125
EPS = 1e-5


def F(ap):
    """f32 view of an f32r tile for elementwise reads."""
    return ap.bitcast(F32)


def build_program(gelu_identity=False):
    nc = bacc.Bacc("TRN2", target_bir_lowering=False, debug=False)

    xT_d = nc.dram_tensor("xT", [2, BL, DIM, N], F32R, kind="ExternalInput").ap()
    w_qk_d = nc.dram_tensor("w_qk", [FI, 128, 2 * DIM], F32R, kind="ExternalInput").ap()
    c_qk_d = nc.dram_tensor("c_qk", [12, 128], F32, kind="ExternalInput").ap()
    w_v_d = nc.dram_tensor("w_v", [FI, 128, DIM], F32R, kind="ExternalInput").ap()
    c_v_d = nc.dram_tensor("c_v", [1, DIM], F32R, kind="ExternalInput").ap()
    w_pr_d = nc.dram_tensor("w_pr", [FI, 128, DIM], BF16, kind="ExternalInput").ap()
    b_pr_d = nc.dram_tensor("b_pr", [FI, 128], F32, kind="ExternalInput").ap()
    w_f1_d = nc.dram_tensor("w_f1", [FI, 128, MLP_HID], F32R, kind="ExternalInput").ap()
    c_f1_d = nc.dram_tensor("c_f1", [1, MLP_HID], F32R, kind="ExternalInput").ap()
    w_f2_d = nc.dram_tensor("w_f2", [24, 128, DIM], BF16, kind="ExternalInput").ap()
    b_f2_d = nc.dram_tensor("b_f2", [FI, 128], F32, kind="ExternalInput").ap()
    w_st_d = nc.dram_tensor("w_st", [3, 128, 128], F32R, kind="ExternalInput").ap()
    pqc_d = nc.dram_tensor("pqc", [128, S], F32, kind="ExternalInput").ap()

    outT_d = nc.dram_tensor("outT", [2, BL, DIM, N], F32, kind="ExternalOutput").ap()
    attn_d = nc.dram_tensor("attn", [2, BL, H, N, N], F32, kind="ExternalOutput").ap()

    x2sp_d = nc.dram_tensor("x2sp", [2, BL, DIM, N], F32R).ap()  # spill

    with tile.TileContext(nc) as tc:
        _body(tc, xT_d, w_qk_d, c_qk_d, w_v_d, c_v_d, w_pr_d, b_pr_d,
              w_f1_d, c_f1_d, w_f2_d, b_f2_d, w_st_d, pqc_d,
              outT_d, attn_d, x2sp_d, gelu_identity)
    nc.compile()
    return nc


def _body(tc, xT_d, w_qk_d, c_qk_d, w_v_d, c_v_d, w_pr_d, b_pr_d,
          w_f1_d, c_f1_d, w_f2_d, b_f2_d, w_st_d, pqc_d,
          outT_d, attn_d, x2sp_d, gelu_identity=False):
    gelu_fn = AF.Identity if gelu_identity else AF.Gelu
    nc = tc.nc
    V = nc.vector
    SC = nc.scalar
    TE = nc.tensor
    G = nc.gpsimd

    import contextlib
    stack = contextlib.ExitStack()
    cst = stack.enter_context(tc.tile_pool(name="cst", bufs=1))

    G.load_library(library_config.attn)

    ident = cst.tile([128, 128], F32)
    make_identity(nc, ident)
    ones_s = cst.tile([128, 128], F32)
    V.memset(ones_s, 1.0)
    ones_f = cst.tile([128, 128], F32R)
    V.tensor_copy(ones_f, ones_s)
    ones_b = cst.tile([128, 128], BF16)
    V.memset(ones_b, 1.0)
    ones_r0 = cst.tile([1, N], F32)
    V.memset(ones_r0, 1.0)
    ones_row = cst.tile([1, N], F32R)
    V.tensor_copy(ones_row, ones_r0)
    eps1 = cst.tile([1, 1], F32)
    V.memset(eps1, EPS)

    def ln_colsums(sq_pool, ab_pool, pspool, src, sqtag, sm_tag, ab_bufs=2, sq_bufs=2):
        """sq + colsum matmuls; returns AB tile with var in [0:N], -mu in [N:2N]
        (Ln/Exp/B-finalize NOT yet applied)."""
        sq = sq_pool.tile([128, FI, N], F32R, tag=sqtag, bufs=sq_bufs, name="sq")
        V.tensor_mul(sq, F(src), F(src))
        S1 = pspool.tile([1, N], F32, tag=sm_tag, name="S1")
        S2 = pspool.tile([1, N], F32, tag=sm_tag, name="S2")
        for f in range(FI):
            TE.matmul(S1, lhsT=ones_f[:, 0:1], rhs=src[:, f, :],
                      start=(f == 0), stop=(f == FI - 1))
        for f in range(FI):
            TE.matmul(S2, lhsT=ones_f[:, 0:1], rhs=sq[:, f, :],
                      start=(f == 0), stop=(f == FI - 1))
        AB = ab_pool.tile([1, 2 * N], F32, tag="ab", bufs=ab_bufs, name="AB")
        m2 = ab_pool.tile([1, N], F32, tag="m2", bufs=1, name="m2")
        V.tensor_scalar(out=AB[:, N:2 * N], in0=S1, scalar1=-1.0 / DIM,
                        scalar2=None, op0=ALU.mult)
        V.tensor_scalar(out=m2, in0=S2, scalar1=1.0 / DIM, scalar2=None, op0=ALU.mult)
        V.tensor_mul(AB[:, 0:N], AB[:, N:2 * N], AB[:, N:2 * N])
        V.tensor_sub(AB[:, 0:N], m2, AB[:, 0:N])
        return AB

    def ln_finish(AB):
        SC.activation(AB[:, 0:N], AB[:, 0:N], AF.Ln, bias=eps1, scale=1.0)
        SC.activation(AB[:, 0:N], AB[:, 0:N], AF.Exp, bias=0.0, scale=-0.5)
        V.tensor_mul(AB[:, N:2 * N], AB[:, N:2 * N], AB[:, 0:N])

    # ---------------- PHASE A ----------------
    with tc.tile_pool(name="wa", bufs=1) as wa, \
         tc.tile_pool(name="pa", bufs=2) as pa, \
         tc.tile_pool(name="sb2", bufs=2) as sb2, \
         tc.tile_pool(name="sb1", bufs=1) as sb1, \
         tc.tile_pool(name="psA", bufs=2, space="PSUM") as psA, \
         tc.tile_pool(name="psQ", bufs=1, space="PSUM") as psQ, \
         tc.tile_pool(name="psT", bufs=1, space="PSUM") as psT:

        w_qk = wa.tile([128, FI, 2 * DIM], F32R)
        nc.sync.dma_start(w_qk, w_qk_d.rearrange("f p j -> p f j"))
        c_qk = wa.tile([128, 12], F32)
        nc.sync.dma_start(c_qk, c_qk_d.rearrange("j p -> p j"))
        w_v = wa.tile([128, FI, DIM], F32R)
        nc.sync.dma_start(w_v, w_v_d.rearrange("f p j -> p f j"))
        c_v = wa.tile([1, DIM], F32R)
        nc.sync.dma_start(c_v, c_v_d)
        w_pr = wa.tile([128, FI, DIM], BF16)
        nc.sync.dma_start(w_pr, w_pr_d.rearrange("f p j -> p f j"))
        b_pr = wa.tile([128, FI], F32)
        nc.sync.dma_start(b_pr, b_pr_d.rearrange("f p -> p f"))
        w_st = wa.tile([128, 3, 128], F32R)
        nc.sync.dma_start(w_st, w_st_d.rearrange("c p m -> p c m"))
        pqc = wa.tile([128, S], F32)
        nc.sync.dma_start(pqc, pqc_d)

        qkT = {}
        vbf = {}
        oT = {}

        def stage1(b, s):
            xT = pa.tile([128, FI, N], F32R, tag="sq", bufs=3, name=f"xT{s}")
            nc.sync.dma_start(xT, xT_d[s, b].rearrange("(f p) t -> p f t", p=128))
            AB = ln_colsums(pa, sb2, psA, xT, "sq", "sm", ab_bufs=2, sq_bufs=3)
            ln_finish(AB)
            ABb = sb2.tile([128, 2 * N], F32, tag="abbc", bufs=1)
            G.partition_broadcast(ABb, AB)
            xn = pa.tile([128, FI, N], F32R, tag="sq", bufs=3)
            V.tensor_mul(xn, F(xT), ABb[:, None, 0:N].to_broadcast((128, FI, N)))
            V.tensor_add(xn, F(xn), ABb[:, None, N:2 * N].to_broadcast((128, FI, N)))
            qk = pa.tile([128, 12, N], F32R, tag=f"qkT{s}", bufs=1, name=f"qkT{s}")
            qkT[s] = qk
            for jc in range(12):
                ps = psA.tile([128, N], F32, tag="sm")
                for f in range(FI):
                    TE.matmul(ps, lhsT=w_qk[:, f, ts(jc, 128)], rhs=xn[:, f, :],
                              start=(f == 0), stop=(f == FI - 1))
                SC.activation(qk[:, jc, :], ps, AF.Identity,
                              bias=c_qk[:, jc:jc + 1], scale=1.0)
            vb = pa.tile([128, 3, DIM], BF16, tag=f"v{s}", bufs=1, name=f"v{s}")
            vbf[s] = vb
            for t3 in range(3):
                tp = TP[t3]
                for hf in range(2):
                    ps = psA.tile([128, 384], F32, tag="sm")
                    for f in range(FI):
                        TE.matmul(ps[0:tp, :], lhsT=xn[:, f, ds(128 * t3, tp)],
                                  rhs=w_v[:, f, ds(384 * hf, 384)],
                                  start=(f == 0), stop=False)
                    TE.matmul(ps[0:tp, :], lhsT=ones_f[0:1, 0:tp],
                              rhs=c_v[:, ds(384 * hf, 384)], start=False, stop=True)
                    V.tensor_copy(vb[0:tp, t3, ds(384 * hf, 384)], ps[0:tp, :])
            oT[s] = pa.tile([128, FI, N], BF16, tag=f"oT{s}", name=f"oT{s}", bufs=1)

        def corr_round(s, hq, ht):
            """q-major corr of head hq + corrT of head ht (opposite parity),
            interleaved so the PE runs both row-groups concurrently."""
            jq = hq // 2
            oq = 64 * (hq % 2)
            ot = 64 * (ht % 2)
            qTq = qkT[s][ds(oq, 64), jq, :]
            kTq = qkT[s][ds(oq, 64), 6 + jq, :]
            qTt = qkT[s][ds(ot, 64), jq, :]
            kTt = qkT[s][ds(ot, 64), 6 + jq, :]
            cq = psQ.tile([128, 3, 512], F32, tag="cq", name="cq")
            ct = psT.tile([128, 3, 512], F32, tag="ct", name="ct")
            for c3 in range(3):
                TE.matmul(cq[0:TP[c3], c3, 0:N], lhsT=qTq[:, ds(128 * c3, TP[c3])],
                          rhs=kTq, start=True, stop=True)
                TE.matmul(ct[0:TP[c3], c3, 0:N], lhsT=kTt[:, ds(128 * c3, TP[c3])],
                          rhs=qTt, start=True, stop=True)
            e = sb2.tile([128, 3, N], F32, tag=f"ec{s}{hq % 2}", bufs=1, name="ec")
            SC.activation(e[:, 0:2, :], cq[:, 0:2, 0:N], AF.Exp, bias=0.0, scale=1.0)
            SC.activation(e[0:64, 2, :], cq[0:64, 2, 0:N], AF.Exp, bias=0.0, scale=1.0)
            sr = sb1.tile([128, 2, 64], F32R, tag=f"st{s}{hq % 2}", name="sr")
            V.tensor_copy(sr[0:64, 0, :], cq[64:128, 0, 0:64])
            V.tensor_copy(sr[64:128, 0, :], cq[0:64, 1, 0:64])
            V.tensor_copy(sr[0:64, 1, :], cq[64:128, 1, 0:64])
            V.tensor_copy(sr[64:128, 1, :], cq[0:64, 2, 0:64])
            eT = sb2.tile([128, 3, N], BF16, tag=f"et{s}{ht % 2}", bufs=1, name="et")
            SC.activation(eT[:, 0:2, :], ct[:, 0:2, 0:N], AF.Exp, bias=0.0, scale=1.0)
            SC.activation(eT[0:64, 2, :], ct[0:64, 2, 0:N], AF.Exp, bias=0.0, scale=1.0)
            cp = sb1.tile([128, 3, S], F32R, tag=f"cp{s}{ht % 2}", name="cp")
            SC.activation(cp[:, 0:2, :], ct[:, 0:2, 64:N], AF.Identity,
                          bias=0.0, scale=1.0)
            SC.activation(cp[0:64, 2, :], ct[0:64, 2, 64:N], AF.Identity,
                          bias=0.0, scale=1.0)
            return e, sr, eT, cp

        def stage2pair(b, hp):
            h0 = 2 * hp
            h1 = 2 * hp + 1
            ec = {}
            st = {}
            et = {}
            cpT = {}
            for s in (0, 1):
                e0, s0, eT1, cp1 = corr_round(s, h0, h1)
                e1, s1_, eT0, cp0 = corr_round(s, h1, h0)
                ec[(s, h0)], st[(s, h0)] = e0, s0
                ec[(s, h1)], st[(s, h1)] = e1, s1_
                et[(s, h0)], cpT[(s, h0)] = eT0, cp0
                et[(s, h1)], cpT[(s, h1)] = eT1, cp1
            # st projections: h0 -> rows 0:64, h1 -> rows 64:128
            qpq = {}
            qpk = {}
            for s in (0, 1):
                qq = sb1.tile([128, S], F32R, tag=f"qpq{s}", name=f"qpq{s}")
                qk_ = sb1.tile([128, S], F32R, tag=f"qpk{s}", name=f"qpk{s}")
                ps = psA.tile([128, 2, S], F32, tag="sm", name="qps")
                for hi, h in enumerate((h0, h1)):
                    for kc in range(3):
                        kk = TP[kc]
                        TE.matmul(ps[:, hi, :], lhsT=w_st[0:kk, kc, :],
                                  rhs=cpT[(s, h)][0:kk, kc, :],
                                  start=(kc == 0), stop=(kc == 2))
                for hi, h in enumerate((h0, h1)):
                    hb = 64 * (h % 2)
                    V.tensor_add(qq[ds(hb, 64), :], ps[0:64, hi, :], pqc[0:64, :])
                    V.tensor_add(qk_[ds(hb, 64), :], ps[64:128, hi, :], pqc[64:128, :])
                qpq[s] = qq
                qpk[s] = qk_
            for d, (qs, ks) in enumerate(((0, 1), (1, 0))):
                ez = {}
                for hi, h in enumerate((h0, h1)):
                    hb = 64 * (h % 2)
                    zt = psA.tile([128, 2, 256], F32, tag="sm", name=f"zt{hi}")
                    for sc in range(2):
                        TE.matmul(zt[:, sc, :],
                                  lhsT=qpk[ks][ds(hb, 64), ts(sc, 128)],
                                  rhs=qpq[qs][ds(hb, 64), :], start=True, stop=True)
                    ezh = sb1.tile([128, 2, S], F32R, tag="ez", bufs=2, name="ez")
                    SC.activation(ezh, zt, AF.Exp, bias=0.0, scale=1.0)
                    ez[h] = ezh
                efp = psA.tile([128, 4, 64], F32, tag="sm", name="efp")
                for hi, h in enumerate((h0, h1)):
                    saf = psA.tile([128, 2, S], F32, tag="sm", name="saf")
                    for sc in range(2):
                        TE.matmul(saf[0:64, 0, :], lhsT=ones_f[:, 0:64],
                                  rhs=ez[h][:, sc, :],
                                  start=(sc == 0), stop=(sc == 1))
                    for sc in range(2):
                        TE.matmul(saf[0:64, 1, :], lhsT=st[(ks, h)][:, sc, :],
                                  rhs=ez[h][:, sc, :],
                                  start=(sc == 0), stop=(sc == 1))
                    ft = sb1.tile([64, S], F32, tag=f"ft{hi}", bufs=2, name="ft")
                    V.reciprocal_approx_fast(out=ft, in_=saf[0:64, 0, :])
                    V.tensor_mul(ft, saf[0:64, 1, :], ft)
                    eft = sb1.tile([64, S], F32, tag=f"eft{hi}", bufs=2, name="eft")
                    SC.activation(eft, ft, AF.Exp, bias=0.0, scale=1.0)
                    V.tensor_mul(et[(qs, h)][0:64, 0, 64:N],
                                 et[(qs, h)][0:64, 0, 64:N], eft)
                    for sc in range(2):
                        TE.transpose(efp[:, 2 * hi + sc, :], eft[:, ts(sc, 128)],
                                     ident[0:64, 0:64])
                for hi, h in enumerate((h0, h1)):
                    e = ec[(qs, h)]
                    V.tensor_mul(e[64:128, 0, 0:64], e[64:128, 0, 0:64],
                                 efp[0:64, 2 * hi, :])
                    V.tensor_mul(e[0:64, 1, 0:64], e[0:64, 1, 0:64],
                                 efp[64:128, 2 * hi, :])
                    V.tensor_mul(e[64:128, 1, 0:64], e[64:128, 1, 0:64],
                                 efp[0:64, 2 * hi + 1, :])
                    V.tensor_mul(e[0:64, 2, 0:64], e[0:64, 2, 0:64],
                                 efp[64:128, 2 * hi + 1, :])
            for h in (h0, h1):
                ou = psA.tile([128, N], F32, tag="sm", name="ou")
                for s in (0, 1):
                    e = ec[(s, h)]
                    eT = et[(s, h)]
                    col = sb1.tile([128, 4], F32, tag=f"col{s}", name="col")
                    for qc in range(3):
                        V.reduce_sum(col[0:TP[qc], qc:qc + 1], e[0:TP[qc], qc, :],
                                     axis=mybir.AxisListType.X)
                    for qc in range(3):
                        G.normalize_recip(e[0:TP[qc], qc, :], e[0:TP[qc], qc, :],
                                          col[0:TP[qc], qc:qc + 1])
                    nc.sync.dma_start(
                        attn_d[s, b, h, 0:256].rearrange("(c p) n -> p c n", p=128),
                        e[:, 0:2, :])
                    nc.sync.dma_start(attn_d[s, b, h, 256:N], e[0:64, 2, :])
                    sbc = psA.tile([128, N], F32, tag="sm", name="sbc")
                    TE.matmul(sbc, lhsT=ones_b, rhs=eT[:, 0, :], start=True, stop=False)
                    TE.matmul(sbc, lhsT=ones_b, rhs=eT[:, 1, :], start=False, stop=False)
                    TE.matmul(sbc, lhsT=ones_b[0:64, :], rhs=eT[0:64, 2, :],
                              start=False, stop=True)
                    rb32 = sb1.tile([128, N], F32, tag="rb32", bufs=2, name="rb32")
                    V.reciprocal_approx_fast(out=rb32, in_=sbc)
                    V.tensor_mul(eT[:, 0:2, :], eT[:, 0:2, :],
                                 rb32[:, None, :].to_broadcast((128, 2, N)))
                    V.tensor_mul(eT[0:64, 2, :], eT[0:64, 2, :], rb32[0:64, :])
                    sb_ = 64 * s
                    for n3 in range(3):
                        kk = TP[n3]
                        TE.matmul(ou[ds(sb_, 64), :],
                                  lhsT=vbf[s][0:kk, n3, ds(64 * h, 64)],
                                  rhs=eT[0:kk, n3, :], start=(n3 == 0), stop=(n3 == 2))
                off = 64 * (h % 2)
                V.tensor_copy(oT[0][ds(off, 64), h // 2, :], ou[0:64, :])
                V.tensor_copy(oT[1][ds(off, 64), h // 2, :], ou[64:128, :])

        def stage25(b, s):
            xT = pa.tile([128, FI, N], F32R, tag="sq", bufs=3, name=f"xTr{s}")
            nc.sync.dma_start(xT, xT_d[s, b].rearrange("(f p) t -> p f t", p=128))
            x2 = pa.tile([128, FI, N], F32R, tag="sq", bufs=3, name="x2T")
            for e6 in range(FI):
                ps = psA.tile([128, N], F32, tag="sm")
                for f in range(FI):
                    TE.matmul(ps, lhsT=w_pr[:, f, ts(e6, 128)], rhs=oT[s][:, f, :],
                              start=(f == 0), stop=(f == FI - 1))
                nc.vector.scalar_tensor_tensor(out=x2[:, e6, :], in0=ps,
                                               scalar=b_pr[:, e6:e6 + 1],
                                               in1=F(xT)[:, e6, :],
                                               op0=ALU.add, op1=ALU.add)
            nc.sync.dma_start(x2sp_d[s, b].rearrange("(f p) t -> p f t", p=128), x2)

        for b in range(BL):
            stage1(b, 0)
            stage1(b, 1)
            for hp in range(H // 2):
                stage2pair(b, hp)
            stage25(b, 0)
            stage25(b, 1)

    # ---------------- PHASE B (MLP) ----------------
    with tc.tile_pool(name="wb", bufs=1) as wb, \
         tc.tile_pool(name="pb", bufs=2) as pb, \
         tc.tile_pool(name="pb1", bufs=1) as pb1, \
         tc.tile_pool(name="psC", bufs=2, space="PSUM") as psC:

        wf1 = []
        wf2 = []
        for g in range(4):
            t1 = wb.tile([128, FI, 768], F32R, tag=f"wf1_{g}", name=f"wf1_{g}")
            nc.sync.dma_start(t1, w_f1_d[:, :, ds(768 * g, 768)].rearrange("f p j -> p f j"))
            wf1.append(t1)
        c_f1 = wb.tile([1, MLP_HID], F32R)
        nc.sync.dma_start(c_f1, c_f1_d)
        for g in range(4):
            t2 = wb.tile([128, FI, DIM], BF16, tag=f"wf2_{g}", name=f"wf2_{g}")
            nc.sync.dma_start(t2, w_f2_d[ds(6 * g, 6)].rearrange("f p j -> p f j"))
            wf2.append(t2)
        b_f2 = wb.tile([128, FI], F32)
        nc.sync.dma_start(b_f2, b_f2_d.rearrange("f p -> p f"))

        def stage3(b, s, AB):
            x2 = pb.tile([128, FI, N], F32R, tag="x2", name="x2")
            nc.sync.dma_start(x2, x2sp_d[s, b].rearrange("(f p) t -> p f t", p=128))
            ABb = pb.tile([128, 2 * N], F32, tag="abbc", bufs=1, name="ABb")
            G.partition_broadcast(ABb, AB)
            xn = pb.tile([128, FI, N], F32R, tag="sq2", name="xn2")
            V.tensor_mul(xn, F(x2), ABb[:, None, 0:N].to_broadcast((128, FI, N)))
            V.tensor_add(xn, F(xn), ABb[:, None, N:2 * N].to_broadcast((128, FI, N)))
            hT = pb1.tile([128, 24, N], BF16, tag="hT", name="hT")
            for g12 in range(12):
                ps = psC.tile([128, 2, 512], F32, tag="h2", name="h2")
                for jj in range(2):
                    j = 2 * g12 + jj
                    for f in range(FI):
                        TE.matmul(ps[:, jj, 0:N], lhsT=wf1[j // 6][:, f, ts(j % 6, 128)],
                                  rhs=xn[:, f, :], start=(f == 0), stop=False)
                    TE.matmul(ps[:, jj, 0:N], lhsT=c_f1[:, ts(j, 128)],
                              rhs=ones_row, start=False, stop=True)
                SC.activation(hT[:, ds(2 * g12, 2), :], ps[:, :, 0:N], gelu_fn,
                              bias=0.0, scale=1.0)
            out_sb = pb.tile([128, FI, N], F32, tag="outT", bufs=1, name="outT")
            for e6 in range(FI):
                ps2 = psC.tile([128, 512], F32, tag="f2", name="f2")
                for j in range(24):
                    TE.matmul(ps2[:, 0:N], lhsT=wf2[j // 6][:, j % 6, ts(e6, 128)],
                              rhs=hT[:, j, :], start=(j == 0), stop=(j == 23))
                nc.vector.scalar_tensor_tensor(out=out_sb[:, e6, :], in0=ps2[:, 0:N],
                                               scalar=b_f2[:, e6:e6 + 1],
                                               in1=F(x2)[:, e6, :],
                                               op0=ALU.add, op1=ALU.add)
            nc.sync.dma_start(outT_d[s, b].rearrange("(f p) t -> p f t", p=128), out_sb)

        pairs = [(b, s) for b in range(BL) for s in (0, 1)]
        for grp in (pairs[:4], pairs[4:]):
            abs_ = []
            for (b, s) in grp:
                x2 = pb.tile([128, FI, N], F32R, tag="sq2", name="x2s")
                nc.sync.dma_start(x2, x2sp_d[s, b].rearrange("(f p) t -> p f t", p=128))
                AB = ln_colsums(pb, pb, psC, x2, "sq2", "f2", ab_bufs=4)
                abs_.append(AB)
            for AB in abs_:
                SC.activation(AB[:, 0:N], AB[:, 0:N], AF.Ln, bias=eps1, scale=1.0)
            for AB in abs_:
                SC.activation(AB[:, 0:N], AB[:, 0:N], AF.Exp, bias=0.0, scale=-0.5)
            for AB in abs_:
                V.tensor_mul(AB[:, N:2 * N], AB[:, N:2 * N], AB[:, 0:N])
            for (b, s), AB in zip(grp, abs_):
                stage3(b, s, AB)

    stack.close()


# ---------------- host side ----------------

def _host_prep(inputs):
    f32 = np.float32
    x_rgb = np.asarray(inputs["x_rgb"], f32)
    x_tir = np.asarray(inputs["x_tir"], f32)
    pos = np.asarray(inputs["pos_emb"], f32)[0, 0]          # [S, N]
    n1w = np.asarray(inputs["norm1_w"], f32)
    n1b = np.asarray(inputs["norm1_b"], f32)
    qkv_w = np.asarray(inputs["qkv_w"], f32)
    proj_w = np.asarray(inputs["proj_w"], f32)
    proj_b = np.asarray(inputs["proj_b"], f32)
    st_q_w = np.asarray(inputs["st_q_w"], f32)
    st_q_b = np.asarray(inputs["st_q_b"], f32)
    st_k_w = np.asarray(inputs["st_k_w"], f32)
    st_k_b = np.asarray(inputs["st_k_b"], f32)
    n2w = np.asarray(inputs["norm2_w"], f32)
    n2b = np.asarray(inputs["norm2_b"], f32)
    fc1_w = np.asarray(inputs["fc1_w"], f32)
    fc1_b = np.asarray(inputs["fc1_b"], f32)
    fc2_w = np.asarray(inputs["fc2_w"], f32)
    fc2_b = np.asarray(inputs["fc2_b"], f32)

    W_qk = ((qkv_w[:2 * DIM] * n1w[None, :]).T).astype(f32).copy()   # [768, 1536]
    W_qk[:, :DIM] *= SCALE
    c_qk = qkv_w[:2 * DIM] @ n1b
    c_qk[:DIM] *= SCALE
    W_v = ((qkv_w[2 * DIM:] * n1w[None, :]).T).astype(f32).copy()
    c_v = qkv_w[2 * DIM:] @ n1b
    W_f1 = ((fc1_w * n2w[None, :]).T).astype(f32).copy()             # [768, 3072]
    c_f1 = fc1_w @ n2b + fc1_b

    # cpT holds the scaled corr slice; pos and biases fold into pqc
    w_st = np.zeros((384, 128), f32)
    w_st[:N, 0:64] = st_q_w.T * MSC
    w_st[:N, 64:128] = st_k_w.T
    pqc = np.zeros((128, S), f32)
    pqc[0:64] = ((pos @ st_q_w.T + st_q_b[None, :]) * MSC).T
    pqc[64:128] = (pos @ st_k_w.T + st_k_b[None, :]).T

    common = {
        "w_qk": W_qk.reshape(FI, 128, 2 * DIM),
        "c_qk": c_qk.reshape(12, 128).astype(f32),
        "w_v": W_v.reshape(FI, 128, DIM),
        "c_v": c_v.reshape(1, DIM).astype(f32),
        "w_pr": proj_w.T.reshape(FI, 128, DIM).astype(ml_dtypes.bfloat16),
        "b_pr": proj_b.reshape(FI, 128).astype(f32),
        "w_f1": W_f1.reshape(FI, 128, MLP_HID),
        "c_f1": c_f1.reshape(1, MLP_HID).astype(f32),
        "w_f2": fc2_w.T.reshape(24, 128, DIM).astype(ml_dtypes.bfloat16),
        "b_f2": fc2_b.reshape(FI, 128).astype(f32),
        "w_st": w_st.reshape(3, 128, 128),
        "pqc": pqc,
    }
    x = np.stack([x_rgb, x_tir])                 # [2, B, N, DIM]
    in_maps = []
    for c in range(NCORES):
        xc = x[:, BL * c:BL * (c + 1)].transpose(0, 1, 3, 2)  # [2, BL, DIM, N]
        m = dict(common)
        m["xT"] = np.ascontiguousarray(xc).astype(f32)
        in_maps.append(m)
    return in_maps


def _host_post(results):
    out = np.empty((2, B, N, DIM), np.float32)
    corr = np.empty((2, B, H, N, N), np.float32)
    for c, r in enumerate(results):
        out[:, BL * c:BL * (c + 1)] = r["outT"].transpose(0, 1, 3, 2)
        corr[:, BL * c:BL * (c + 1)] = r["attn"]
    return out[0], out[1], corr[0], corr[1]


_CACHED_NC = None


def kernel(**inputs):
    global _CACHED_NC
    if _CACHED_NC is None:
        _CACHED_NC = build_program()
    in_maps = _host_prep(inputs)
    res = bass_utils.run_bass_kernel_spmd(_CACHED_NC, in_maps,
                                          core_ids=list(range(NCORES)))
    return _host_post(res.results)


# revision 28
# speedup vs baseline: 1.0408x; 1.0408x over previous
"""Trainium2 Bass kernel for nn_CMABlock (CMA dual-stream transformer block).

Self-contained: hardcodes shapes/sharding. Data-parallel over batch across
8 NeuronCores (4 batch elements per core, both rgb/tir streams on each).

v2: head-paired stage 2 (opposite PE row-groups run concurrently), GPSIMD
softmax normalization, approx reciprocals, grouped LN-stats to avoid ACT
table-set thrashing, aggressive SBUF tag sharing.
"""
import sys
sys.path.insert(0, '/opt/trn_rl_repo')

import numpy as np
import ml_dtypes

import concourse.bass as bass
import concourse.tile as tile
from concourse import mybir
from concourse import bacc
from concourse import bass_utils
from concourse import library_config
from concourse.bass import ts, ds
from concourse.masks import make_identity

F32 = mybir.dt.float32
F32R = mybir.dt.float32r
BF16 = mybir.dt.bfloat16
AF = mybir.ActivationFunctionType
ALU = mybir.AluOpType

B, N, DIM, H = 32, 320, 768, 12
DH = DIM // H          # 64
T = 64                 # template tokens
S = N - T              # 256 search tokens
MATCH = 64
MLP_HID = 4 * DIM      # 3072
NCORES = 8
BL = B // NCORES       # 4 batch per core
FI = DIM // 128        # 6 feature chunks
TP = [128, 128, 64]    # token chunk sizes (320)
SCALE = DH ** -0.5     # 0.125
MSC = MATCH ** -0.5    # 0.125
EPS = 1e-5


def F(ap):
    """f32 view of an f32r tile for elementwise reads."""
    return ap.bitcast(F32)


def build_program(gelu_identity=False):
    nc = bacc.Bacc("TRN2", target_bir_lowering=False, debug=False)

    xT_d = nc.dram_tensor("xT", [2, BL, DIM, N], F32R, kind="ExternalInput").ap()
    w_qk_d = nc.dram_tensor("w_qk", [FI, 128, 2 * DIM], F32R, kind="ExternalInput").ap()
    c_qk_d = nc.dram_tensor("c_qk", [12, 128], F32, kind="ExternalInput").ap()
    w_v_d = nc.dram_tensor("w_v", [FI, 128, DIM], F32R, kind="ExternalInput").ap()
    c_v_d = nc.dram_tensor("c_v", [1, DIM], F32R, kind="ExternalInput").ap()
    w_pr_d = nc.dram_tensor("w_pr", [FI, 128, DIM], BF16, kind="ExternalInput").ap()
    b_pr_d = nc.dram_tensor("b_pr", [FI, 128], F32, kind="ExternalInput").ap()
    w_f1_d = nc.dram_tensor("w_f1", [FI, 128, MLP_HID], F32R, kind="ExternalInput").ap()
    c_f1_d = nc.dram_tensor("c_f1", [1, MLP_HID], F32R, kind="ExternalInput").ap()
    w_f2_d = nc.dram_tensor("w_f2", [24, 128, DIM], BF16, kind="ExternalInput").ap()
    b_f2_d = nc.dram_tensor("b_f2", [FI, 128], F32, kind="ExternalInput").ap()
    w_st_d = nc.dram_tensor("w_st", [3, 128, 128], F32R, kind="ExternalInput").ap()
    pqc_d = nc.dram_tensor("pqc", [128, S], F32, kind="ExternalInput").ap()

    outT_d = nc.dram_tensor("outT", [2, BL, DIM, N], F32, kind="ExternalOutput").ap()
    attn_d = nc.dram_tensor("attn", [2, BL, H, N, N], F32, kind="ExternalOutput").ap()

    x2sp_d = nc.dram_tensor("x2sp", [2, BL, DIM, N], F32R).ap()  # spill

    with tile.TileContext(nc) as tc:
        _body(tc, xT_d, w_qk_d, c_qk_d, w_v_d, c_v_d, w_pr_d, b_pr_d,
              w_f1_d, c_f1_d, w_f2_d, b_f2_d, w_st_d, pqc_d,
              outT_d, attn_d, x2sp_d, gelu_identity)
    nc.compile()
    return nc


def _body(tc, xT_d, w_qk_d, c_qk_d, w_v_d, c_v_d, w_pr_d, b_pr_d,
          w_f1_d, c_f1_d, w_f2_d, b_f2_d, w_st_d, pqc_d,
          outT_d, attn_d, x2sp_d, gelu_identity=False):
    gelu_fn = AF.Identity if gelu_identity else AF.Gelu
    nc = tc.nc
    V = nc.vector
    SC = nc.scalar
    TE = nc.tensor
    G = nc.gpsimd

    import contextlib
    stack = contextlib.ExitStack()
    cst = stack.enter_context(tc.tile_pool(name="cst", bufs=1))

    G.load_library(library_config.attn)

    ident = cst.tile([128, 128], F32)
    make_identity(nc, ident)
    ones_s = cst.tile([128, 128], F32)
    V.memset(ones_s, 1.0)
    ones_f = cst.tile([128, 128], F32R)
    V.tensor_copy(ones_f, ones_s)
    ones_b = cst.tile([128, 128], BF16)
    V.memset(ones_b, 1.0)
    ones_r0 = cst.tile([1, N], F32)
    V.memset(ones_r0, 1.0)
    ones_row = cst.tile([1, N], F32R)
    V.tensor_copy(ones_row, ones_r0)
    eps1 = cst.tile([1, 1], F32)
    V.memset(eps1, EPS)

    def ln_colsums(sq_pool, ab_pool, pspool, src, sqtag, sm_tag, ab_bufs=2, sq_bufs=2):
        """sq + colsum matmuls; returns AB tile with var in [0:N], -mu in [N:2N]
        (Ln/Exp/B-finalize NOT yet applied)."""
        sq = sq_pool.tile([128, FI, N], F32R, tag=sqtag, bufs=sq_bufs, name="sq")
        V.tensor_mul(sq, F(src), F(src))
        S1 = pspool.tile([1, N], F32, tag=sm_tag, name="S1")
        S2 = pspool.tile([1, N], F32, tag=sm_tag, name="S2")
        for f in range(FI):
            TE.matmul(S1, lhsT=ones_f[:, 0:1], rhs=src[:, f, :],
                      start=(f == 0), stop=(f == FI - 1))
        for f in range(FI):
            TE.matmul(S2, lhsT=ones_f[:, 0:1], rhs=sq[:, f, :],
                      start=(f == 0), stop=(f == FI - 1))
        AB = ab_pool.tile([1, 2 * N], F32, tag="ab", bufs=ab_bufs, name="AB")
        m2 = ab_pool.tile([1, N], F32, tag="m2", bufs=1, name="m2")
        V.tensor_scalar(out=AB[:, N:2 * N], in0=S1, scalar1=-1.0 / DIM,
                        scalar2=None, op0=ALU.mult)
        V.tensor_scalar(out=m2, in0=S2, scalar1=1.0 / DIM, scalar2=None, op0=ALU.mult)
        V.tensor_mul(AB[:, 0:N], AB[:, N:2 * N], AB[:, N:2 * N])
        V.tensor_sub(AB[:, 0:N], m2, AB[:, 0:N])
        return AB

    def ln_finish(AB):
        SC.activation(AB[:, 0:N], AB[:, 0:N], AF.Ln, bias=eps1, scale=1.0)
        SC.activation(AB[:, 0:N], AB[:, 0:N], AF.Exp, bias=0.0, scale=-0.5)
        V.tensor_mul(AB[:, N:2 * N], AB[:, N:2 * N], AB[:, 0:N])

    # ---------------- PHASE A ----------------
    with tc.tile_pool(name="wa", bufs=1) as wa, \
         tc.tile_pool(name="pa", bufs=2) as pa, \
         tc.tile_pool(name="sb2", bufs=2) as sb2, \
         tc.tile_pool(name="sb1", bufs=1) as sb1, \
         tc.tile_pool(name="psA", bufs=2, space="PSUM") as psA, \
         tc.tile_pool(name="psQ", bufs=1, space="PSUM") as psQ, \
         tc.tile_pool(name="psT", bufs=1, space="PSUM") as psT:

        w_qk = wa.tile([128, FI, 2 * DIM], F32R)
        nc.sync.dma_start(w_qk, w_qk_d.rearrange("f p j -> p f j"))
        c_qk = wa.tile([128, 12], F32)
        nc.sync.dma_start(c_qk, c_qk_d.rearrange("j p -> p j"))
        w_v = wa.tile([128, FI, DIM], F32R)
        nc.sync.dma_start(w_v, w_v_d.rearrange("f p j -> p f j"))
        c_v = wa.tile([1, DIM], F32R)
        nc.sync.dma_start(c_v, c_v_d)
        w_pr = wa.tile([128, FI, DIM], BF16)
        nc.sync.dma_start(w_pr, w_pr_d.rearrange("f p j -> p f j"))
        b_pr = wa.tile([128, FI], F32)
        nc.sync.dma_start(b_pr, b_pr_d.rearrange("f p -> p f"))
        w_st = wa.tile([128, 3, 128], F32R)
        nc.sync.dma_start(w_st, w_st_d.rearrange("c p m -> p c m"))
        pqc = wa.tile([128, S], F32)
        nc.sync.dma_start(pqc, pqc_d)

        qkT = {}
        vbf = {}
        oT = {}

        def stage1(b, s):
            xT = pa.tile([128, FI, N], F32R, tag="sq", bufs=3, name=f"xT{s}")
            nc.sync.dma_start(xT, xT_d[s, b].rearrange("(f p) t -> p f t", p=128))
            AB = ln_colsums(pa, sb2, psA, xT, "sq", "sm", ab_bufs=2, sq_bufs=3)
            ln_finish(AB)
            ABb = sb2.tile([128, 2 * N], F32, tag="abbc", bufs=1)
            G.partition_broadcast(ABb, AB)
            xn = pa.tile([128, FI, N], F32R, tag="sq", bufs=3)
            V.tensor_mul(xn, F(xT), ABb[:, None, 0:N].to_broadcast((128, FI, N)))
            V.tensor_add(xn, F(xn), ABb[:, None, N:2 * N].to_broadcast((128, FI, N)))
            qk = pa.tile([128, 12, N], F32R, tag=f"qkT{s}", bufs=1, name=f"qkT{s}")
            qkT[s] = qk
            for jc in range(12):
                ps = psA.tile([128, N], F32, tag="sm")
                for f in range(FI):
                    TE.matmul(ps, lhsT=w_qk[:, f, ts(jc, 128)], rhs=xn[:, f, :],
                              start=(f == 0), stop=(f == FI - 1))
                SC.activation(qk[:, jc, :], ps, AF.Identity,
                              bias=c_qk[:, jc:jc + 1], scale=1.0)
            vb = pa.tile([128, 3, DIM], BF16, tag=f"v{s}", bufs=1, name=f"v{s}")
            vbf[s] = vb
            for t3 in range(3):
                tp = TP[t3]
                for hf in range(2):
                    ps = psA.tile([128, 384], F32, tag="sm")
                    for f in range(FI):
                        TE.matmul(ps[0:tp, :], lhsT=xn[:, f, ds(128 * t3, tp)],
                                  rhs=w_v[:, f, ds(384 * hf, 384)],
                                  start=(f == 0), stop=False)
                    TE.matmul(ps[0:tp, :], lhsT=ones_f[0:1, 0:tp],
                              rhs=c_v[:, ds(384 * hf, 384)], start=False, stop=True)
                    V.tensor_copy(vb[0:tp, t3, ds(384 * hf, 384)], ps[0:tp, :])
            oT[s] = pa.tile([128, FI, N], BF16, tag=f"oT{s}", name=f"oT{s}", bufs=1)

        def corr_round(s, hq, ht):
            """q-major corr of head hq + corrT of head ht (opposite parity),
            interleaved so the PE runs both row-groups concurrently."""
            jq = hq // 2
            oq = 64 * (hq % 2)
            ot = 64 * (ht % 2)
            qTq = qkT[s][ds(oq, 64), jq, :]
            kTq = qkT[s][ds(oq, 64), 6 + jq, :]
            qTt = qkT[s][ds(ot, 64), jq, :]
            kTt = qkT[s][ds(ot, 64), 6 + jq, :]
            cq = psQ.tile([128, 3, 512], F32, tag="cq", name="cq")
            ct = psT.tile([128, 3, 512], F32, tag="ct", name="ct")
            for c3 in range(3):
                TE.matmul(cq[0:TP[c3], c3, 0:N], lhsT=qTq[:, ds(128 * c3, TP[c3])],
                          rhs=kTq, start=True, stop=True)
                TE.matmul(ct[0:TP[c3], c3, 0:N], lhsT=kTt[:, ds(128 * c3, TP[c3])],
                          rhs=qTt, start=True, stop=True)
            e = sb2.tile([128, 3, N], F32, tag=f"ec{s}{hq % 2}", bufs=1, name="ec")
            SC.activation(e, cq[:, :, 0:N], AF.Exp, bias=0.0, scale=1.0)
            sr = sb1.tile([128, 2, 64], F32R, tag=f"st{s}{hq % 2}", name="sr")
            V.tensor_copy(sr[0:64, 0:2, :], cq[64:128, 0:2, 0:64])
            V.tensor_copy(sr[64:128, 0:2, :], cq[0:64, 1:3, 0:64])
            eT = sb2.tile([128, 3, N], BF16, tag=f"et{s}{ht % 2}", bufs=1, name="et")
            SC.activation(eT, ct[:, :, 0:N], AF.Exp, bias=0.0, scale=1.0)
            cp = sb1.tile([128, 3, S], F32R, tag=f"cp{s}{ht % 2}", name="cp")
            SC.activation(cp[:, 0:2, :], ct[:, 0:2, 64:N], AF.Identity,
                          bias=0.0, scale=1.0)
            SC.activation(cp[0:64, 2, :], ct[0:64, 2, 64:N], AF.Identity,
                          bias=0.0, scale=1.0)
            return e, sr, eT, cp

        def stage2pair(b, hp):
            h0 = 2 * hp
            h1 = 2 * hp + 1
            ec = {}
            st = {}
            et = {}
            cpT = {}
            for s in (0, 1):
                e0, s0, eT1, cp1 = corr_round(s, h0, h1)
                e1, s1_, eT0, cp0 = corr_round(s, h1, h0)
                ec[(s, h0)], st[(s, h0)] = e0, s0
                ec[(s, h1)], st[(s, h1)] = e1, s1_
                et[(s, h0)], cpT[(s, h0)] = eT0, cp0
                et[(s, h1)], cpT[(s, h1)] = eT1, cp1
            # st projections: h0 -> rows 0:64, h1 -> rows 64:128
            qpq = {}
            qpk = {}
            for s in (0, 1):
                qq = sb1.tile([128, S], F32R, tag=f"qpq{s}", name=f"qpq{s}")
                qk_ = sb1.tile([128, S], F32R, tag=f"qpk{s}", name=f"qpk{s}")
                ps = psA.tile([128, 2, S], F32, tag="sm", name="qps")
                for hi, h in enumerate((h0, h1)):
                    for kc in range(3):
                        kk = TP[kc]
                        TE.matmul(ps[:, hi, :], lhsT=w_st[0:kk, kc, :],
                                  rhs=cpT[(s, h)][0:kk, kc, :],
                                  start=(kc == 0), stop=(kc == 2))
                for hi, h in enumerate((h0, h1)):
                    hb = 64 * (h % 2)
                    V.tensor_add(qq[ds(hb, 64), :], ps[0:64, hi, :], pqc[0:64, :])
                    V.tensor_add(qk_[ds(hb, 64), :], ps[64:128, hi, :], pqc[64:128, :])
                qpq[s] = qq
                qpk[s] = qk_
            for d, (qs, ks) in enumerate(((0, 1), (1, 0))):
                ez = {}
                for hi, h in enumerate((h0, h1)):
                    hb = 64 * (h % 2)
                    zt = psA.tile([128, 2, 256], F32, tag="sm", name=f"zt{hi}")
                    for sc in range(2):
                        TE.matmul(zt[:, sc, :],
                                  lhsT=qpk[ks][ds(hb, 64), ts(sc, 128)],
                                  rhs=qpq[qs][ds(hb, 64), :], start=True, stop=True)
                    ezh = sb1.tile([128, 2, S], F32R, tag="ez", bufs=2, name="ez")
                    SC.activation(ezh, zt, AF.Exp, bias=0.0, scale=1.0)
                    ez[h] = ezh
                efp = psA.tile([128, 4, 64], F32, tag="sm", name="efp")
                for hi, h in enumerate((h0, h1)):
                    saf = psA.tile([128, 2, S], F32, tag="sm", name="saf")
                    for sc in range(2):
                        TE.matmul(saf[0:64, 0, :], lhsT=ones_f[:, 0:64],
                                  rhs=ez[h][:, sc, :],
                                  start=(sc == 0), stop=(sc == 1))
                    for sc in range(2):
                        TE.matmul(saf[0:64, 1, :], lhsT=st[(ks, h)][:, sc, :],
                                  rhs=ez[h][:, sc, :],
                                  start=(sc == 0), stop=(sc == 1))
                    ft = sb1.tile([64, S], F32, tag=f"ft{hi}", bufs=2, name="ft")
                    V.reciprocal_approx_fast(out=ft, in_=saf[0:64, 0, :])
                    V.tensor_mul(ft, saf[0:64, 1, :], ft)
                    eft = sb1.tile([64, S], F32, tag=f"eft{hi}", bufs=2, name="eft")
                    SC.activation(eft, ft, AF.Exp, bias=0.0, scale=1.0)
                    V.tensor_mul(et[(qs, h)][0:64, 0, 64:N],
                                 et[(qs, h)][0:64, 0, 64:N], eft)
                    for sc in range(2):
                        TE.transpose(efp[:, 2 * hi + sc, :], eft[:, ts(sc, 128)],
                                     ident[0:64, 0:64])
                for hi, h in enumerate((h0, h1)):
                    e = ec[(qs, h)]
                    V.tensor_mul(e[64:128, 0:2, 0:64], e[64:128, 0:2, 0:64],
                                 efp[0:64, ds(2 * hi, 2), :])
                    V.tensor_mul(e[0:64, 1:3, 0:64], e[0:64, 1:3, 0:64],
                                 efp[64:128, ds(2 * hi, 2), :])
            for h in (h0, h1):
                ou = psA.tile([128, N], F32, tag="sm", name="ou")
                for s in (0, 1):
                    e = ec[(s, h)]
                    eT = et[(s, h)]
                    col = sb1.tile([128, 4], F32, tag=f"col{s}", name="col")
                    V.reduce_sum(col[:, 0:3], e, axis=mybir.AxisListType.X)
                    for qc in range(3):
                        G.normalize_recip(e[0:TP[qc], qc, :], e[0:TP[qc], qc, :],
                                          col[0:TP[qc], qc:qc + 1])
                    nc.sync.dma_start(
                        attn_d[s, b, h, 0:256].rearrange("(c p) n -> p c n", p=128),
                        e[:, 0:2, :])
                    nc.sync.dma_start(attn_d[s, b, h, 256:N], e[0:64, 2, :])
                    sbc = psA.tile([128, N], F32, tag="sm", name="sbc")
                    TE.matmul(sbc, lhsT=ones_b, rhs=eT[:, 0, :], start=True, stop=False)
                    TE.matmul(sbc, lhsT=ones_b, rhs=eT[:, 1, :], start=False, stop=False)
                    TE.matmul(sbc, lhsT=ones_b[0:64, :], rhs=eT[0:64, 2, :],
                              start=False, stop=True)
                    rb32 = sb1.tile([128, N], F32, tag="rb32", bufs=2, name="rb32")
                    V.reciprocal_approx_fast(out=rb32, in_=sbc)
                    V.tensor_mul(eT, eT, rb32[:, None, :].to_broadcast((128, 3, N)))
                    sb_ = 64 * s
                    for n3 in range(3):
                        kk = TP[n3]
                        TE.matmul(ou[ds(sb_, 64), :],
                                  lhsT=vbf[s][0:kk, n3, ds(64 * h, 64)],
                                  rhs=eT[0:kk, n3, :], start=(n3 == 0), stop=(n3 == 2))
                off = 64 * (h % 2)
                V.tensor_copy(oT[0][ds(off, 64), h // 2, :], ou[0:64, :])
                V.tensor_copy(oT[1][ds(off, 64), h // 2, :], ou[64:128, :])

        def stage25(b, s):
            xT = pa.tile([128, FI, N], F32R, tag="sq", bufs=3, name=f"xTr{s}")
            nc.sync.dma_start(xT, xT_d[s, b].rearrange("(f p) t -> p f t", p=128))
            x2 = pa.tile([128, FI, N], F32R, tag="sq", bufs=3, name="x2T")
            for e6 in range(FI):
                ps = psA.tile([128, N], F32, tag="sm")
                for f in range(FI):
                    TE.matmul(ps, lhsT=w_pr[:, f, ts(e6, 128)], rhs=oT[s][:, f, :],
                              start=(f == 0), stop=(f == FI - 1))
                nc.vector.scalar_tensor_tensor(out=x2[:, e6, :], in0=ps,
                                               scalar=b_pr[:, e6:e6 + 1],
                                               in1=F(xT)[:, e6, :],
                                               op0=ALU.add, op1=ALU.add)
            nc.sync.dma_start(x2sp_d[s, b].rearrange("(f p) t -> p f t", p=128), x2)

        for b in range(BL):
            stage1(b, 0)
            stage1(b, 1)
            for hp in range(H // 2):
                stage2pair(b, hp)
            stage25(b, 0)
            stage25(b, 1)

    # ---------------- PHASE B (MLP) ----------------
    with tc.tile_pool(name="wb", bufs=1) as wb, \
         tc.tile_pool(name="pb", bufs=2) as pb, \
         tc.tile_pool(name="pb1", bufs=1) as pb1, \
         tc.tile_pool(name="psC", bufs=2, space="PSUM") as psC:

        wf1 = []
        wf2 = []
        for g in range(4):
            t1 = wb.tile([128, FI, 768], F32R, tag=f"wf1_{g}", name=f"wf1_{g}")
            nc.sync.dma_start(t1, w_f1_d[:, :, ds(768 * g, 768)].rearrange("f p j -> p f j"))
            wf1.append(t1)
        c_f1 = wb.tile([1, MLP_HID], F32R)
        nc.sync.dma_start(c_f1, c_f1_d)
        for g in range(4):
            t2 = wb.tile([128, FI, DIM], BF16, tag=f"wf2_{g}", name=f"wf2_{g}")
            nc.sync.dma_start(t2, w_f2_d[ds(6 * g, 6)].rearrange("f p j -> p f j"))
            wf2.append(t2)
        b_f2 = wb.tile([128, FI], F32)
        nc.sync.dma_start(b_f2, b_f2_d.rearrange("f p -> p f"))

        def stage3(b, s, AB):
            x2 = pb.tile([128, FI, N], F32R, tag="x2", name="x2")
            nc.sync.dma_start(x2, x2sp_d[s, b].rearrange("(f p) t -> p f t", p=128))
            ABb = pb.tile([128, 2 * N], F32, tag="abbc", bufs=1, name="ABb")
            G.partition_broadcast(ABb, AB)
            xn = pb.tile([128, FI, N], F32R, tag="sq2", name="xn2")
            V.tensor_mul(xn, F(x2), ABb[:, None, 0:N].to_broadcast((128, FI, N)))
            V.tensor_add(xn, F(xn), ABb[:, None, N:2 * N].to_broadcast((128, FI, N)))
            hT = pb1.tile([128, 24, N], BF16, tag="hT", name="hT")
            for g12 in range(12):
                ps = psC.tile([128, 2, 512], F32, tag="h2", name="h2")
                for jj in range(2):
                    j = 2 * g12 + jj
                    for f in range(FI):
                        TE.matmul(ps[:, jj, 0:N], lhsT=wf1[j // 6][:, f, ts(j % 6, 128)],
                                  rhs=xn[:, f, :], start=(f == 0), stop=False)
                    TE.matmul(ps[:, jj, 0:N], lhsT=c_f1[:, ts(j, 128)],
                              rhs=ones_row, start=False, stop=True)
                SC.activation(hT[:, ds(2 * g12, 2), :], ps[:, :, 0:N], gelu_fn,
                              bias=0.0, scale=1.0)
            out_sb = pb.tile([128, FI, N], F32, tag="outT", bufs=1, name="outT")
            for e6 in range(FI):
                ps2 = psC.tile([128, 512], F32, tag="f2", name="f2")
                for j in range(24):
                    TE.matmul(ps2[:, 0:N], lhsT=wf2[j // 6][:, j % 6, ts(e6, 128)],
                              rhs=hT[:, j, :], start=(j == 0), stop=(j == 23))
                nc.vector.scalar_tensor_tensor(out=out_sb[:, e6, :], in0=ps2[:, 0:N],
                                               scalar=b_f2[:, e6:e6 + 1],
                                               in1=F(x2)[:, e6, :],
                                               op0=ALU.add, op1=ALU.add)
            nc.sync.dma_start(outT_d[s, b].rearrange("(f p) t -> p f t", p=128), out_sb)

        pairs = [(b, s) for b in range(BL) for s in (0, 1)]
        for grp in (pairs[:4], pairs[4:]):
            abs_ = []
            for (b, s) in grp:
                x2 = pb.tile([128, FI, N], F32R, tag="sq2", name="x2s")
                nc.sync.dma_start(x2, x2sp_d[s, b].rearrange("(f p) t -> p f t", p=128))
                AB = ln_colsums(pb, pb, psC, x2, "sq2", "f2", ab_bufs=4)
                abs_.append(AB)
            for AB in abs_:
                SC.activation(AB[:, 0:N], AB[:, 0:N], AF.Ln, bias=eps1, scale=1.0)
            for AB in abs_:
                SC.activation(AB[:, 0:N], AB[:, 0:N], AF.Exp, bias=0.0, scale=-0.5)
            for AB in abs_:
                V.tensor_mul(AB[:, N:2 * N], AB[:, N:2 * N], AB[:, 0:N])
            for (b, s), AB in zip(grp, abs_):
                stage3(b, s, AB)

    stack.close()


# ---------------- host side ----------------

def _host_prep(inputs):
    f32 = np.float32
    x_rgb = np.asarray(inputs["x_rgb"], f32)
    x_tir = np.asarray(inputs["x_tir"], f32)
    pos = np.asarray(inputs["pos_emb"], f32)[0, 0]          # [S, N]
    n1w = np.asarray(inputs["norm1_w"], f32)
    n1b = np.asarray(inputs["norm1_b"], f32)
    qkv_w = np.asarray(inputs["qkv_w"], f32)
    proj_w = np.asarray(inputs["proj_w"], f32)
    proj_b = np.asarray(inputs["proj_b"], f32)
    st_q_w = np.asarray(inputs["st_q_w"], f32)
    st_q_b = np.asarray(inputs["st_q_b"], f32)
    st_k_w = np.asarray(inputs["st_k_w"], f32)
    st_k_b = np.asarray(inputs["st_k_b"], f32)
    n2w = np.asarray(inputs["norm2_w"], f32)
    n2b = np.asarray(inputs["norm2_b"], f32)
    fc1_w = np.asarray(inputs["fc1_w"], f32)
    fc1_b = np.asarray(inputs["fc1_b"], f32)
    fc2_w = np.asarray(inputs["fc2_w"], f32)
    fc2_b = np.asarray(inputs["fc2_b"], f32)

    W_qk = ((qkv_w[:2 * DIM] * n1w[None, :]).T).astype(f32).copy()   # [768, 1536]
    W_qk[:, :DIM] *= SCALE
    c_qk = qkv_w[:2 * DIM] @ n1b
    c_qk[:DIM] *= SCALE
    W_v = ((qkv_w[2 * DIM:] * n1w[None, :]).T).astype(f32).copy()
    c_v = qkv_w[2 * DIM:] @ n1b
    W_f1 = ((fc1_w * n2w[None, :]).T).astype(f32).copy()             # [768, 3072]
    c_f1 = fc1_w @ n2b + fc1_b

    # cpT holds the scaled corr slice; pos and biases fold into pqc
    w_st = np.zeros((384, 128), f32)
    w_st[:N, 0:64] = st_q_w.T * MSC
    w_st[:N, 64:128] = st_k_w.T
    pqc = np.zeros((128, S), f32)
    pqc[0:64] = ((pos @ st_q_w.T + st_q_b[None, :]) * MSC).T
    pqc[64:128] = (pos @ st_k_w.T + st_k_b[None, :]).T

    common = {
        "w_qk": W_qk.reshape(FI, 128, 2 * DIM),
        "c_qk": c_qk.reshape(12, 128).astype(f32),
        "w_v": W_v.reshape(FI, 128, DIM),
        "c_v": c_v.reshape(1, DIM).astype(f32),
        "w_pr": proj_w.T.reshape(FI, 128, DIM).astype(ml_dtypes.bfloat16),
        "b_pr": proj_b.reshape(FI, 128).astype(f32),
        "w_f1": W_f1.reshape(FI, 128, MLP_HID),
        "c_f1": c_f1.reshape(1, MLP_HID).astype(f32),
        "w_f2": fc2_w.T.reshape(24, 128, DIM).astype(ml_dtypes.bfloat16),
        "b_f2": fc2_b.reshape(FI, 128).astype(f32),
        "w_st": w_st.reshape(3, 128, 128),
        "pqc": pqc,
    }
    x = np.stack([x_rgb, x_tir])                 # [2, B, N, DIM]
    in_maps = []
    for c in range(NCORES):
        xc = x[:, BL * c:BL * (c + 1)].transpose(0, 1, 3, 2)  # [2, BL, DIM, N]
        m = dict(common)
        m["xT"] = np.ascontiguousarray(xc).astype(f32)
        in_maps.append(m)
    return in_maps


def _host_post(results):
    out = np.empty((2, B, N, DIM), np.float32)
    corr = np.empty((2, B, H, N, N), np.float32)
    for c, r in enumerate(results):
        out[:, BL * c:BL * (c + 1)] = r["outT"].transpose(0, 1, 3, 2)
        corr[:, BL * c:BL * (c + 1)] = r["attn"]
    return out[0], out[1], corr[0], corr[1]


_CACHED_NC = None


def kernel(**inputs):
    global _CACHED_NC
    if _CACHED_NC is None:
        _CACHED_NC = build_program()
    in_maps = _host_prep(inputs)
    res = bass_utils.run_bass_kernel_spmd(_CACHED_NC, in_maps,
                                          core_ids=list(range(NCORES)))
    return _host_post(res.results)
